# revision 49
# baseline (speedup 1.0000x reference)
"""Trainium2 Bass kernel for nn_Attention_146028888114.

Full attention block: LN -> QKV -> per-head QK-LN -> RoPE -> SDPA -> out-proj.
B=2, S=2048, D=1024, H=16, HD=64, fp32 (f32r matmuls).

Sharding: DP over batch (2 groups of 4 cores) x TP over heads (4 heads/core).
Each core computes a partial out-projection (its 4 heads' contribution); the
host sums the 4 partials per batch (the unshard/reduce step).

v2 layout: phase 1 computes LN + x-transposes (persisted for all tiles) and
only the K/V projections + K rope; the Q projection/rope runs lazily inside
the phase-2 attention loop (chunks 0/1 pre-built during phase 1) so its PE
work fills the gaps left while the Activation engine grinds through the
softmax exp (the phase-2 bottleneck). Weights are DMA'd directly as f32r (no
staging copies), every rstd is exp(-0.5*ln(var+eps)) so the whole kernel
shares one ACT table, and the softmax denominator broadcast uses the Pool
engine's partition_broadcast instead of a DRAM round-trip.
"""

import sys

sys.path.insert(0, "/opt/trn_rl_repo")

import numpy as np

import concourse.bass as bass
import concourse.tile as tile
from concourse import bacc, mybir

# All ACT functions this kernel uses (Ln, Exp, Copy, Square, Identity) live
# together in the "natural_log_exp_and_others" table; the default greedy
# table-selection pass still thrashes between the exp-only and ln-only sets
# (a 1.3us reload each time), so restrict the choice to the one shared set.
# Positions are preserved: act_func_set_id indexes the original list.
from concourse.hw_specs import get_activation_tables as _get_act_tables


def _nlx_only_tables(arch):
    out = {}
    for name, funcs in _get_act_tables(arch).items():
        keep = name in ("natural_log_exp_and_others", "sqrt_and_others")
        out[name] = funcs if keep else set()
    return out


bacc.get_activation_tables = _nlx_only_tables

from concourse.alu_op_type import AluOpType
from concourse.bass_utils import run_bass_kernel_spmd
from concourse.masks import make_identity

B, S, D = 2, 2048, 1024
H, HD = 16, 64
EPS = 1e-6
ROPE_BASE = 10000.0

NCORES = 8
GROUPS = 4            # cores per batch group (TP degree)
HLOC = H // GROUPS    # heads per core
P = 128
ST = S // P           # 16 s-tiles
KC = D // P           # 8 k-chunks of the contraction
NQ = HLOC * HD        # 256 q columns per core
NKV = 2 * HLOC * HD   # 512 k+v columns per core
SQW = 512             # sq chunk width
SQC = S // SQW        # 4 sq chunks
# v cols + ones cols at 64.. (denominator trick); padded to a 32 multiple:
# matmul silently zeroes output rows past M=64 when M isn't 32-aligned.
VW = 96

DT = mybir.dt
F32 = DT.float32
F32R = DT.float32r
AF = mybir.ActivationFunctionType
X_AXIS = mybir.AxisListType.X


def build_program(use_bias_qkv: bool, use_bias_out: bool, repeat: int = 1):
    nc = bacc.Bacc("TRN2", target_bir_lowering=False, debug=False, num_devices=NCORES)

    x_d = nc.dram_tensor("x", [S, D], F32, kind="ExternalInput")
    # weights / tables arrive pre-tiled (and f32r-typed) from the host
    wkv_d = nc.dram_tensor("wkv", [P, KC * NKV], F32R, kind="ExternalInput")
    wq_d = nc.dram_tensor("wq", [P, KC * NQ], F32R, kind="ExternalInput")
    bkv_d = nc.dram_tensor("bkv", [1, NKV], F32R, kind="ExternalInput")
    bq_d = nc.dram_tensor("bq", [1, NQ], F32R, kind="ExternalInput")
    wout_d = nc.dram_tensor("wout", [P, 2 * D], F32R, kind="ExternalInput")
    bout_d = nc.dram_tensor("bout", [1, D], F32R, kind="ExternalInput")
    tab_d = {nm: nc.dram_tensor(nm, [P, ST * HD], F32, kind="ExternalInput")
             for nm in ("cq", "sq", "ck", "sk")}
    out_d = nc.dram_tensor("out", [S, D], F32, kind="ExternalOutput")

    ones1_d = nc.inline_tensor(np.ones((1, P), dtype=np.float32), name="ones1")

    with tile.TileContext(nc) as tc:
        with tc.tile_pool(name="const", bufs=1) as cpool, \
             tc.tile_pool(name="data", bufs=1) as dpool:
            # --- persistent data tiles ---
            xr_all = dpool.tile([P, ST, KC, P], F32R, tag="xra")
            kT = dpool.tile([P, 2, S], F32R, tag="kT")
            v_all = dpool.tile([P, ST, HLOC, VW], F32R, tag="v")
            attnN = dpool.tile([P, 2, S], F32R, tag="attnN")
            wkv_r = dpool.tile([P, KC, NKV], F32R, tag="wkv")
            wq_r = dpool.tile([P, KC, NQ], F32R, tag="wq")
            wout_r = dpool.tile([P, 2, D], F32R, tag="wout")

            # --- constants ---
            ident = cpool.tile([P, P], F32)
            make_identity(nc, ident[:])
            ident_r = cpool.tile([P, P], F32R)
            nc.vector.tensor_copy(ident_r[:], ident[:])

            eps_t = cpool.tile([P, 1], F32)
            nc.vector.memset(eps_t[:], EPS)

            # weight DMAs are deferred until after the first x-tile DMA so
            # phase 1's critical path wins the DMA queues; rope tables are
            # streamed per-tile (saves 16KB of SBUF)
            def emit_weight_dmas():
                nc.sync.dma_start(wkv_r[:].rearrange("p a b -> p (a b)"), wkv_d[:])
                nc.sync.dma_start(wq_r[:].rearrange("p a b -> p (a b)"), wq_d[:])
                nc.sync.dma_start(wout_r[:].rearrange("p a b -> p (a b)"), wout_d[:])

            if use_bias_qkv or use_bias_out:
                ones1_f = cpool.tile([1, P], F32)
                nc.sync.dma_start(ones1_f[:], ones1_d[:])
                ones1r = cpool.tile([1, P], F32R)
                nc.vector.tensor_copy(ones1r[:], ones1_f[:])
            if use_bias_qkv:
                bkv_r = cpool.tile([1, NKV], F32R)
                nc.sync.dma_start(bkv_r[:], bkv_d[:])
                bq_r = cpool.tile([1, NQ], F32R)
                nc.sync.dma_start(bq_r[:], bq_d[:])
            if use_bias_out:
                bo_r = cpool.tile([1, D], F32R)
                nc.sync.dma_start(bo_r[:], bout_d[:])

            # ones columns of v (denominator trick): one ACT broadcast copy
            onescol_f = cpool.tile([P, 1], F32)
            nc.vector.memset(onescol_f[:], 1.0)
            nc.scalar.activation(
                v_all[:, :, :, HD:VW],
                onescol_f[:, :, None, None].to_broadcast((P, ST, HLOC, VW - HD)),
                AF.Copy,
            )

            with tc.tile_pool(name="qt", bufs=2) as qtp, \
                 tc.tile_pool(name="ph2q", bufs=2) as ph2q, \
                 tc.tile_pool(name="ph2qs", bufs=4) as ph2qs, \
                 tc.tile_pool(name="mq_ps", bufs=2, space="PSUM") as mqp:

              def emit_lazy_q(j):
                # project + LN + rope the 4 tiles of q chunk j
                qTj = qtp.tile([P, 2, SQW], F32R, tag="qTj")
                for tt in range(SQW // P):
                    t = j * (SQW // P) + tt
                    q_ps = mqp.tile([P, NQ], F32, tag="mq")
                    first = True
                    if use_bias_qkv:
                        nc.tensor.matmul(
                            q_ps[:], lhsT=ones1r[:], rhs=bq_r[:],
                            start=True, stop=False)
                        first = False
                    for k in range(KC):
                        nc.tensor.matmul(
                            q_ps[:], lhsT=xr_all[:, t, k, :], rhs=wq_r[:, k, :],
                            start=first, stop=(k == KC - 1))
                        first = False

                    q_g = q_ps[:].rearrange("p (h d) -> p h d", h=HLOC)
                    sums = ph2qs.tile([P, HLOC], F32, tag="sums")
                    nc.vector.reduce_sum(sums[:], q_g, axis=X_AXIS)
                    sq = ph2q.tile([P, HLOC, HD], F32, tag="sq")
                    nc.scalar.square(
                        sq[:].rearrange("p h d -> p (h d)"), q_ps[:])
                    sumsq = ph2qs.tile([P, HLOC], F32, tag="sumsq")
                    nc.vector.reduce_sum(sumsq[:], sq[:], axis=X_AXIS)
                    mean4 = ph2qs.tile([P, HLOC], F32, tag="mean4")
                    nc.vector.tensor_scalar_mul(mean4[:], sums[:], 1.0 / HD)
                    m2 = ph2qs.tile([P, HLOC], F32, tag="m2")
                    nc.vector.tensor_tensor(m2[:], mean4[:], mean4[:], op=AluOpType.mult)
                    var4 = ph2qs.tile([P, HLOC], F32, tag="var4")
                    nc.vector.scalar_tensor_tensor(
                        var4[:], sumsq[:], 1.0 / HD, m2[:],
                        op0=AluOpType.mult, op1=AluOpType.subtract)
                    lnv4 = ph2qs.tile([P, HLOC], F32, tag="lnv4")
                    nc.scalar.activation(lnv4[:], var4[:], AF.Ln, bias=eps_t[:])
                    rstd4 = ph2qs.tile([P, HLOC], F32, tag="rstd4")
                    nc.scalar.activation(rstd4[:], lnv4[:], AF.Exp, scale=-0.5)
                    qn = ph2q.tile([P, HLOC, HD], F32R, tag="qn")
                    nc.vector.tensor_tensor(
                        qn[:], q_g, mean4[:, :, None].to_broadcast((P, HLOC, HD)),
                        op=AluOpType.subtract)
                    nc.vector.tensor_tensor(
                        qn[:], qn[:], rstd4[:, :, None].to_broadcast((P, HLOC, HD)),
                        op=AluOpType.mult)

                    cq_t = ph2q.tile([P, HD], F32, tag="cqt", bufs=3)
                    nc.sync.dma_start(cq_t[:], tab_d["cq"][:, t * HD:(t + 1) * HD])
                    sq_t = ph2q.tile([P, HD], F32, tag="sqt", bufs=3)
                    nc.sync.dma_start(sq_t[:], tab_d["sq"][:, t * HD:(t + 1) * HD])
                    ct = cq_t[:, None, :].to_broadcast((P, HLOC, HD))
                    s_lo = sq_t[:, None, 0:32].to_broadcast((P, HLOC, 32))
                    s_hi = sq_t[:, None, 32:64].to_broadcast((P, HLOC, 32))
                    r2 = ph2q.tile([P, HLOC, HD], F32R, tag="r2")
                    nc.gpsimd.tensor_tensor(
                        r2[:, :, 0:32], qn[:, :, 32:64], s_lo, op=AluOpType.mult)
                    nc.gpsimd.tensor_tensor(
                        r2[:, :, 32:64], qn[:, :, 0:32], s_hi, op=AluOpType.mult)
                    r3 = ph2q.tile([P, HLOC, HD], F32R, tag="r3")
                    nc.vector.tensor_tensor(r3[:], qn[:], ct, op=AluOpType.mult)
                    tq = mqp.tile([P, 2, P], F32R, tag="mq")
                    for hp in range(2):
                        nc.tensor.matmul(
                            tq[:, hp, :],
                            lhsT=r3[:, 2 * hp:2 * hp + 2, :].rearrange("p h d -> p (h d)"),
                            rhs=ident_r[:], is_transpose=True,
                            start=True, stop=False)
                        nc.tensor.matmul(
                            tq[:, hp, :],
                            lhsT=r2[:, 2 * hp:2 * hp + 2, :].rearrange("p h d -> p (h d)"),
                            rhs=ident_r[:], is_transpose=True,
                            start=False, stop=True)
                    nc.scalar.activation(
                        qTj[:, :, tt * P:(tt + 1) * P], tq[:], AF.Copy)
                return qTj

              for _rep in range(repeat):
               # ---------------- Phase 1: LN + transposes + K/V proj + K rope --
               qts = {}
               with tc.tile_pool(name="ph1", bufs=3) as ph1, \
                   tc.tile_pool(name="ph1s", bufs=4) as ph1s, \
                   tc.tile_pool(name="tp_ps", bufs=2, space="PSUM") as tpp, \
                   tc.tile_pool(name="tk_ps", bufs=2, space="PSUM") as tkp, \
                   tc.tile_pool(name="kv_ps", bufs=2, space="PSUM") as kvp:
                # prefetch the first x tiles ahead of the bulk weight DMAs
                x_pre = {}
                for t in range(3):
                    xt = ph1.tile([P, D], F32, tag="x", name=f"xpre{t}")
                    nc.sync.dma_start(xt[:], x_d[t * P:(t + 1) * P, :])
                    x_pre[t] = xt
                if _rep == 0:
                    emit_weight_dmas()
                for t in range(ST):
                    if t in x_pre:
                        x_t = x_pre.pop(t)
                    else:
                        x_t = ph1.tile([P, D], F32, tag="x")
                        nc.sync.dma_start(x_t[:], x_d[t * P:(t + 1) * P, :])

                    # input LN stats
                    st1 = ph1s.tile([P, 2, 6], F32, tag="st1")
                    nc.vector.bn_stats(st1[:, 0, :], x_t[:, 0:512])
                    nc.vector.bn_stats(st1[:, 1, :], x_t[:, 512:1024])
                    mv = ph1s.tile([P, 2], F32, tag="mv")
                    nc.vector.bn_aggr(mv[:], st1[:])
                    lnv = ph1s.tile([P, 1], F32, tag="lnv")
                    nc.scalar.activation(lnv[:], mv[:, 1:2], AF.Ln, bias=eps_t[:])
                    rstd = ph1s.tile([P, 1], F32, tag="rstd")
                    nc.scalar.activation(rstd[:], lnv[:], AF.Exp, scale=-0.5)
                    # x normalize split across Pool and DVE halves
                    xn = ph1.tile([P, D], F32R, tag="xn", bufs=3)
                    nc.gpsimd.tensor_scalar(
                        xn[:, 0:512], x_t[:, 0:512], scalar1=mv[:, 0:1],
                        scalar2=rstd[:],
                        op0=AluOpType.subtract, op1=AluOpType.mult,
                    )
                    nc.vector.tensor_scalar(
                        xn[:, 512:1024], x_t[:, 512:1024], scalar1=mv[:, 0:1],
                        scalar2=rstd[:],
                        op0=AluOpType.subtract, op1=AluOpType.mult,
                    )

                    # transpose xn into the persistent xr_all[t]
                    xr = xr_all[:, t]
                    for g in range(2):
                        tp = tpp.tile([P, 4, P], F32R, tag="tp")
                        for q4 in range(4):
                            kc = 4 * g + q4
                            nc.tensor.transpose(
                                tp[:, q4, :], xn[:, kc * P:(kc + 1) * P], ident_r[:])
                        nc.scalar.activation(
                            xr[:, 4 * g:4 * g + 4, :], tp[:], AF.Copy)

                    # K/V projection: one 512-wide accumulation chain
                    kv_ps = kvp.tile([P, NKV], F32, tag="kv")
                    first = True
                    if use_bias_qkv:
                        nc.tensor.matmul(
                            kv_ps[:], lhsT=ones1r[:], rhs=bkv_r[:],
                            start=True, stop=False)
                        first = False
                    for k in range(KC):
                        nc.tensor.matmul(
                            kv_ps[:], lhsT=xr[:, k, :], rhs=wkv_r[:, k, :],
                            start=first, stop=(k == KC - 1))
                        first = False

                    # v copy (natural, f32r) into persistent v_all
                    nc.scalar.activation(
                        v_all[:, t, :, 0:HD],
                        kv_ps[:, NQ:NKV].rearrange("p (h d) -> p h d", h=HLOC),
                        AF.Copy,
                    )

                    # per-head K LayerNorm
                    k_g = kv_ps[:, 0:NQ].rearrange("p (h d) -> p h d", h=HLOC)
                    sums = ph1s.tile([P, HLOC], F32, tag="sums")
                    nc.vector.reduce_sum(sums[:], k_g, axis=X_AXIS)
                    sq = ph1.tile([P, NQ], F32, tag="sq", bufs=2)
                    nc.scalar.square(sq[:], kv_ps[:, 0:NQ])
                    sumsq = ph1s.tile([P, HLOC], F32, tag="sumsq")
                    nc.vector.reduce_sum(
                        sumsq[:], sq[:].rearrange("p (h d) -> p h d", h=HLOC),
                        axis=X_AXIS)
                    mean4 = ph1s.tile([P, HLOC], F32, tag="mean4")
                    nc.vector.tensor_scalar_mul(mean4[:], sums[:], 1.0 / HD)
                    m2 = ph1s.tile([P, HLOC], F32, tag="m2")
                    nc.vector.tensor_tensor(m2[:], mean4[:], mean4[:], op=AluOpType.mult)
                    var4 = ph1s.tile([P, HLOC], F32, tag="var4")
                    nc.vector.scalar_tensor_tensor(
                        var4[:], sumsq[:], 1.0 / HD, m2[:],
                        op0=AluOpType.mult, op1=AluOpType.subtract)
                    lnv4 = ph1s.tile([P, HLOC], F32, tag="lnv4")
                    nc.scalar.activation(lnv4[:], var4[:], AF.Ln, bias=eps_t[:])
                    rstd4 = ph1s.tile([P, HLOC], F32, tag="rstd4")
                    nc.scalar.activation(rstd4[:], lnv4[:], AF.Exp, scale=-0.5)
                    kn = ph1.tile([P, HLOC, HD], F32R, tag="kn", bufs=2)
                    nc.vector.tensor_tensor(
                        kn[:], k_g, mean4[:, :, None].to_broadcast((P, HLOC, HD)),
                        op=AluOpType.subtract)
                    nc.gpsimd.tensor_tensor(
                        kn[:], kn[:], rstd4[:, :, None].to_broadcast((P, HLOC, HD)),
                        op=AluOpType.mult)

                    # RoPE on Pool/DVE: out = kn*cos_t + swap(kn)*sgn_sin_t
                    ck_t = ph1.tile([P, HD], F32, tag="ckt", bufs=3)
                    nc.sync.dma_start(ck_t[:], tab_d["ck"][:, t * HD:(t + 1) * HD])
                    sk_t = ph1.tile([P, HD], F32, tag="skt", bufs=3)
                    nc.sync.dma_start(sk_t[:], tab_d["sk"][:, t * HD:(t + 1) * HD])
                    ct = ck_t[:, None, :].to_broadcast((P, HLOC, HD))
                    s_lo = sk_t[:, None, 0:32].to_broadcast((P, HLOC, 32))
                    s_hi = sk_t[:, None, 32:64].to_broadcast((P, HLOC, 32))
                    r2 = ph1.tile([P, HLOC, HD], F32R, tag="r2", bufs=2)
                    nc.gpsimd.tensor_tensor(
                        r2[:, :, 0:32], kn[:, :, 32:64], s_lo, op=AluOpType.mult)
                    nc.gpsimd.tensor_tensor(
                        r2[:, :, 32:64], kn[:, :, 0:32], s_hi, op=AluOpType.mult)
                    r3 = ph1.tile([P, HLOC, HD], F32R, tag="r3", bufs=2)
                    nc.vector.tensor_tensor(r3[:], kn[:], ct, op=AluOpType.mult)
                    # r4 = r3 + r2 fused into the transposes via PSUM accumulation
                    tq = tkp.tile([P, 2, P], F32R, tag="tk")
                    for hp in range(2):
                        nc.tensor.matmul(
                            tq[:, hp, :],
                            lhsT=r3[:, 2 * hp:2 * hp + 2, :].rearrange("p h d -> p (h d)"),
                            rhs=ident_r[:], is_transpose=True,
                            start=True, stop=False)
                        nc.tensor.matmul(
                            tq[:, hp, :],
                            lhsT=r2[:, 2 * hp:2 * hp + 2, :].rearrange("p h d -> p (h d)"),
                            rhs=ident_r[:], is_transpose=True,
                            start=False, stop=True)
                    nc.scalar.activation(
                        kT[:, :, t * P:(t + 1) * P], tq[:], AF.Copy)
                    # pre-build the first two q chunks inside phase 1: their
                    # PE work fills phase-1 slack and phase 2 starts hot
                    if t == 3:
                        qts[0] = emit_lazy_q(0)
                    elif t == 7:
                        qts[1] = emit_lazy_q(1)

               # ---------------- Phase 2: attention + out-projection ----------
               with tc.tile_pool(name="pt", bufs=4) as ptp, \
                   tc.tile_pool(name="ph3", bufs=1) as ph3, \
                   tc.tile_pool(name="ob", bufs=2) as obp, \
                   tc.tile_pool(name="st_ps", bufs=2, space="PSUM") as stp, \
                   tc.tile_pool(name="attn_ps", bufs=2, space="PSUM") as atp:

                def emit_outproj(jj):
                    for m in range(SQW // P):
                        row = jj * SQW + m * P
                        for lo_i in range(2):
                            lo = lo_i * 512
                            o_ps = mqp.tile([P, 512], F32, tag="mq")
                            first = True
                            if use_bias_out:
                                nc.tensor.matmul(
                                    o_ps[:], lhsT=ones1r[:], rhs=bo_r[:, lo:lo + 512],
                                    start=True, stop=False)
                                first = False
                            for hp in range(2):
                                nc.tensor.matmul(
                                    o_ps[:],
                                    lhsT=attnN[:, hp, row:row + P],
                                    rhs=wout_r[:, hp, lo:lo + 512],
                                    start=first, stop=(hp == 1))
                                first = False
                            ob = obp.tile([P, 512], F32, tag="ob")
                            nc.vector.tensor_copy(ob[:], o_ps[:])
                            nc.sync.dma_start(out_d[row:row + P, lo:lo + 512], ob[:])

                for j in range(SQC):
                    qTj = qts.pop(j)
                    # ---- attention for chunk j
                    attn_sb = [None] * HLOC
                    for hp in range(2):
                        attn_ps = []
                        for h2 in range(2):
                            a_ps = atp.tile([VW, SQW], F32, tag="attn")
                            attn_ps.append(a_ps)
                        for i in range(ST):
                            sT = stp.tile([P, 2, SQW], F32, tag="st")
                            for h2 in range(2):
                                fl = h2 * HD
                                nc.tensor.matmul(
                                    sT[:, h2, :],
                                    lhsT=kT[fl:fl + HD, hp, i * P:(i + 1) * P],
                                    rhs=qTj[fl:fl + HD, hp, :],
                                    start=True, stop=True)
                            pT = ptp.tile([P, 2, SQW], F32R, tag="pt")
                            nc.scalar.activation(pT[:], sT[:], AF.Exp, scale=0.125)
                            for h2 in range(2):
                                h = 2 * hp + h2
                                nc.tensor.matmul(
                                    attn_ps[h2][:],
                                    lhsT=v_all[:, i, h, :],
                                    rhs=pT[:, h2, :],
                                    start=(i == 0), stop=(i == ST - 1))
                        for h2 in range(2):
                            h = 2 * hp + h2
                            a_sb = ph3.tile([HD + 1, SQW], F32, tag=f"asb{h}")
                            nc.vector.tensor_copy(a_sb[:], attn_ps[h2][0:HD + 1, :])
                            attn_sb[h] = a_sb
                        # normalize this pair now (divide by the denominator
                        # row, write attnN) so the last chunk's out-proj isn't
                        # serialized behind both pairs. partition_broadcast
                        # replicates the tile's partition 0 on hardware (the
                        # AP partition offset is ignored), so each head's
                        # reciprocal gets its own partition-0 tile.
                        for h2 in range(2):
                            h = 2 * hp + h2
                            fl = h2 * HD
                            rec1 = ph3.tile([1, SQW], F32, tag="rec", bufs=2)
                            nc.vector.reciprocal(rec1[:], attn_sb[h][HD:HD + 1, :])
                            bc = ph3.tile([HD, SQW], F32, tag="bc", bufs=2)
                            nc.gpsimd.partition_broadcast(bc[:], rec1[0:1, :])
                            nc.vector.tensor_tensor(
                                attnN[fl:fl + HD, hp, j * SQW:(j + 1) * SQW],
                                attn_sb[h][0:HD, :], bc[:], op=AluOpType.mult)
                        # queue chunk j+2's lazy-q between the two head
                        # pairs: PE picks it up whenever exp is the limiter
                        if hp == 0 and j + 2 < SQC:
                            qts[j + 2] = emit_lazy_q(j + 2)

                    # out-proj of the previous chunk: inputs long ready, so PE
                    # starts immediately while this chunk's attention runs
                    if j >= 1:
                        emit_outproj(j - 1)

                emit_outproj(SQC - 1)

    nc.compile()
    return nc


_PROGRAM_CACHE = {}


def _get_program(use_bias_qkv, use_bias_out):
    key = (use_bias_qkv, use_bias_out)
    if key not in _PROGRAM_CACHE:
        _PROGRAM_CACHE[key] = build_program(*key)
    return _PROGRAM_CACHE[key]


def _rope_tables(q_scale, k_scale):
    inv_freq = 1.0 / ROPE_BASE ** (np.arange(0, HD, 2, dtype=np.float32) / HD)
    t = np.arange(S, dtype=np.float32)
    freqs = np.einsum("i,j->ij", t, inv_freq)
    emb = np.concatenate((freqs, freqs), axis=-1)          # [S, HD]
    cos = np.cos(emb).astype(np.float32)
    sin = np.sin(emb).astype(np.float32)
    sgnsin = sin.copy()
    sgnsin[:, 0:HD // 2] *= -1.0
    swap = lambda v: np.concatenate((v[HD // 2:], v[:HD // 2]))
    tabs = {}
    for nm, sc in (("q", q_scale), ("k", k_scale)):
        tabs["c" + nm] = np.ascontiguousarray(cos * sc[None, :])
        tabs["s" + nm] = np.ascontiguousarray(sgnsin * swap(sc)[None, :])
    return tabs


def _tile_rows(a):
    """[S-like rows, W] -> [P, (rows/P) * W] partition-tiled layout."""
    r, w = a.shape
    return np.ascontiguousarray(
        a.reshape(r // P, P, w).transpose(1, 0, 2).reshape(P, (r // P) * w))


def make_in_maps(x, w_qkv, b_qkv, w_out, b_out, ln_scale, ln_bias, q_scale, k_scale):
    tabs = _rope_tables(q_scale, k_scale)
    tabs_tiled = {nm: _tile_rows(v) for nm, v in tabs.items()}
    wq, wk, wv = w_qkv[:, 0:D], w_qkv[:, D:2 * D], w_qkv[:, 2 * D:3 * D]
    bq, bk, bv = b_qkv[0:D], b_qkv[D:2 * D], b_qkv[2 * D:3 * D]
    in_maps = []
    for c in range(NCORES):
        b = c // GROUPS
        h0 = (c % GROUPS) * HLOC
        cols = slice(h0 * HD, (h0 + HLOC) * HD)
        wkv_raw = np.concatenate([wk[:, cols], wv[:, cols]], axis=1)
        wq_raw = wq[:, cols]
        bkv_c = np.concatenate([bk[cols], bv[cols]]) + ln_bias @ wkv_raw
        bq_c = bq[cols] + ln_bias @ wq_raw
        wkv_c = ln_scale[:, None] * wkv_raw
        wq_c = ln_scale[:, None] * wq_raw
        wout_c = w_out[cols, :]
        bout_c = b_out if (c % GROUPS) == 0 else np.zeros_like(b_out)
        in_maps.append({
            "x": np.ascontiguousarray(x[b]),
            "wkv": _tile_rows(wkv_c),
            "wq": _tile_rows(wq_c),
            "bkv": bkv_c.reshape(1, -1).astype(np.float32),
            "bq": bq_c.reshape(1, -1).astype(np.float32),
            "wout": _tile_rows(wout_c),
            "bout": bout_c.reshape(1, -1).astype(np.float32),
            **tabs_tiled,
        })
    return in_maps


def kernel(x, w_qkv, b_qkv, w_out, b_out, ln_scale, ln_bias, q_scale, k_scale):
    args = [np.asarray(a, dtype=np.float32) for a in
            (x, w_qkv, b_qkv, w_out, b_out, ln_scale, ln_bias, q_scale, k_scale)]
    in_maps = make_in_maps(*args)
    use_bias_qkv = any(np.any(m["bkv"]) or np.any(m["bq"]) for m in in_maps)
    use_bias_out = any(np.any(m["bout"]) for m in in_maps)
    nc = _get_program(use_bias_qkv, use_bias_out)

    res = run_bass_kernel_spmd(nc, in_maps, core_ids=list(range(NCORES)))

    out = np.zeros((B, S, D), dtype=np.float32)
    for c in range(NCORES):
        out[c // GROUPS] += res.results[c]["out"]
    return out
